# revision 91
# baseline (speedup 1.0000x reference)
"""Trainium2 Bass kernel for a single masked attention head.

Problem: B=8, S=2048, DIM_IN=768, DIM_K=DIM_V=64.
  q = query @ W_q.T + b_q ; k = key @ W_k.T + b_k ; v = value @ W_v.T + b_v
  scores = (q @ k.T) / 8 ; scores[mask] = -inf ; out = softmax(scores) @ v

Sharding: data-parallel over batch — one batch element per NeuronCore (8 cores).

Key ideas:
  * Everything stays feature-major so the softmax reduction never lands on
    the partition axis: scores are computed transposed, S.T[b,a], the key
    mask becomes a per-partition bias folded into the ACT exp, and the
    softmax denominator comes from an extra ones-column appended to V
    (carried through the vaug transpose as an extra row of vTx).
    No row-max is needed (scores are O(+-8); fp32 exp is safe; masked
    lanes get bias -1e4 so their exp underflows to exactly 0).
  * Masked keys (~half of them) are dropped entirely: the host computes a
    valid-first permutation of key indices from the tiny [S] mask (pure
    metadata), and the kernel gathers only CAP=1152 key/value rows via
    indirect DMA. Padding slots point at masked rows and carry bias -1e4,
    so they contribute exactly 0. The serialized SWDGE gather stream is
    the front-half pacer, so everything else is arranged around it.
  * X/W travel as bf16 (host-side cast): halves HBM traffic, PE feeder
    transposes run 1 cycle/col with 2-byte LDWEIGHTS, and the 16-bit DMA
    XBAR transpose becomes available - used for vaug, where it is off
    the critical path and clear of the gather stream's DMA-semaphore
    recycle chain (XBAR transfers are queue-synchronous and race other
    XBARs across queues, so bulk X transposes stay on the PE). All
    matmuls accumulate in fp32 PSUM.
  * Scores+exp for each key chunk-group start as soon as that group is
    projected (interleaved into the gather stream); PV waits only for
    vaug at the end. The exp chain on the scalar engine and the gather
    stream on the Pool queue run concurrently, both near-critical.
"""

import numpy as np

S = 2048
DIN = 768
DK = 64
NI = DIN // 128  # feature chunks
CAP = 1152       # compacted key/value capacity (valid keys ~1024+-23)
MASK_NEG = -10000.0

_CACHE = {}


def build_nc(s=S, cap=CAP):
    import concourse.bacc as bacc
    import concourse.bass as bass
    import concourse.mybir as mybir
    import concourse.tile as tile
    from concourse.masks import make_identity

    f32 = mybir.dt.float32
    f32r = mybir.dt.float32r
    bf16 = mybir.dt.bfloat16
    i32 = mybir.dt.int32
    na = s // 512
    nbk = cap // 128
    CG = 3  # k/v chunks per projection group

    nc = bacc.Bacc("TRN2", target_bir_lowering=False, debug=False)

    xq_d = nc.dram_tensor("xq", [s, DIN], bf16, kind="ExternalInput")
    xk_d = nc.dram_tensor("xk", [s, DIN], bf16, kind="ExternalInput")
    xv_d = nc.dram_tensor("xv", [s, DIN], bf16, kind="ExternalInput")
    idx_d = nc.dram_tensor("kvidx", [128, nbk], i32, kind="ExternalInput")
    mb_d = nc.dram_tensor("maskb", [128, nbk], f32, kind="ExternalInput")
    wq_d = nc.dram_tensor("wq", [DK, DIN], bf16, kind="ExternalInput")
    wk_d = nc.dram_tensor("wk", [DK, DIN], bf16, kind="ExternalInput")
    wv_d = nc.dram_tensor("wv", [DK, DIN], bf16, kind="ExternalInput")
    bq_d = nc.dram_tensor("bq", [1, DK], f32, kind="ExternalInput")
    bk_d = nc.dram_tensor("bk", [1, DK], f32, kind="ExternalInput")
    bv_d = nc.dram_tensor("bv", [1, DK], f32, kind="ExternalInput")
    out_d = nc.dram_tensor("out", [s, DK], f32, kind="ExternalOutput")

    with tile.TileContext(nc) as tc:
        with (
            tc.tile_pool(name="const", bufs=1) as cp,
            tc.tile_pool(name="kvstage", bufs=20) as kvp,
            tc.tile_pool(name="xt", bufs=2) as xtp,
            tc.tile_pool(name="pt", bufs=na * nbk + 4) as ptp,
            tc.tile_pool(name="osb", bufs=2) as osp,
            tc.tile_pool(name="ps_tpr", bufs=3, space="PSUM") as ps_tpr,
            tc.tile_pool(name="ps_proj", bufs=1, space="PSUM") as ps_proj,
            tc.tile_pool(name="ps_st", bufs=2, space="PSUM") as ps_st,
            tc.tile_pool(name="ps_ot", bufs=2, space="PSUM") as ps_ot,
        ):
            # ---- metadata first (gathers wait on idxs), then weights ----
            idxs = cp.tile([128, nbk], i32)
            nc.sync.dma_start(idxs[:], idx_d.ap())
            maskb = cp.tile([128, nbk], f32)
            nc.sync.dma_start(maskb[:], mb_d.ap())

            w_sbs = {}
            b_sbs = {}
            for name, w_d, b_d in (
                ("q", wq_d, bq_d), ("k", wk_d, bk_d), ("v", wv_d, bv_d),
            ):
                w_sb = cp.tile([DK, DIN], bf16, tag=f"wload_{name}")
                nc.sync.dma_start(w_sb[:], w_d.ap())
                b_sb = cp.tile([1, DK], f32, tag=f"bld_{name}")
                nc.sync.dma_start(b_sb[:], b_d.ap())
                w_sbs[name] = w_sb
                b_sbs[name] = b_sb

            # ---- K/V row gathers (SWDGE, serialized — the pacer) ----
            staged = {"k": [], "v": []}
            for name, x_d in (("k", xk_d), ("v", xv_d)):
                for c in range(nbk):
                    x_sb = kvp.tile([128, DIN], bf16, tag="kvload")
                    nc.gpsimd.indirect_dma_start(
                        out=x_sb[:],
                        out_offset=None,
                        in_=x_d.ap(),
                        in_offset=bass.IndirectOffsetOnAxis(
                            ap=idxs[:, c:c + 1], axis=0,
                        ),
                    )
                    staged[name].append(x_sb)

            # ---- identities + weight transposes on the (idle) PE ----
            identF = cp.tile([128, 128], f32, tag="identF")
            make_identity(nc, identF[:])
            identB = cp.tile([128, 128], bf16, tag="identB")
            nc.vector.tensor_copy(identB[:], identF[:])
            identR = cp.tile([128, 128], f32r, tag="identR")
            nc.vector.tensor_copy(identR[:], identF[:])

            wts = {}
            biases = {}
            one_c = cp.tile([1, 1], f32)
            nc.vector.memset(one_c[:], 1.0)
            for name in ("q", "k", "v"):
                w_sb = w_sbs[name]
                wt = cp.tile([128, NI, DK], bf16, tag=f"wt_{name}")
                tp = ps_tpr.tile([128, 512], bf16, tag="tpr")
                for i in range(NI):
                    nc.tensor.transpose(
                        tp[:, i * DK:(i + 1) * DK],
                        w_sb[:, i * 128:(i + 1) * 128], identB[:DK, :DK],
                    )
                nc.vector.tensor_copy(
                    wt[:], tp[:, 0:NI * DK].rearrange("p (i e) -> p i e", i=NI)
                )
                wts[name] = wt
                bp = ps_tpr.tile([DK, 1], f32, tag="tpr")
                nc.tensor.matmul(bp[:], b_sbs[name][:], one_c[:])
                bt = cp.tile([DK, 1], f32, tag=f"b_{name}")
                nc.vector.tensor_copy(bt[:], bp[:])
                biases[name] = bt

            # ---- projection targets; vTx carries the denominator rows ----
            qT = cp.tile([DK, s], bf16)
            kT = cp.tile([DK, cap], bf16)
            vTx = cp.tile([80, cap], bf16)
            nc.vector.memset(vTx[64:80, :], 0.0)
            nc.vector.memset(vTx[64:65, :], 1.0)
            vaug = cp.tile([128, nbk, 80], bf16)

            def project(name, dst, w, mov_of_i):
                pj = ps_proj.tile([DK, 512], f32, tag="proj")
                for i in range(NI):
                    nc.tensor.matmul(
                        pj[:, 0:w], wts[name][:, i, :], mov_of_i(i),
                        start=(i == 0), stop=(i == NI - 1),
                    )
                nc.vector.tensor_scalar_add(dst, pj[:, 0:w], biases[name][:])

            def transpose_project(name, rows, dst, w, pump=None,
                                  copy_eng=None):
                """PE-transpose row-chunk tiles, then project into dst."""
                xt = xtp.tile([128, NI, 512], bf16, tag="xt")
                nch = w // 128
                eng = copy_eng if copy_eng is not None else nc.any
                for g in range(0, NI, 2):
                    tp = ps_tpr.tile([128, 2, 512], bf16, tag="tpr")
                    for u in range(2):
                        for ss in range(nch):
                            nc.tensor.transpose(
                                tp[:, u, ss * 128:(ss + 1) * 128],
                                rows[ss][:, (g + u) * 128:(g + u + 1) * 128],
                                identB[:],
                            )
                    eng.tensor_copy(xt[:, g:g + 2, 0:w], tp[:, :, 0:w])
                    if pump is not None:
                        pump()
                project(name, dst, w, lambda i: xt[:, i, 0:w])

            def project_kv(name, g0, pump=None, copy_eng=None):
                gn = min(CG, nbk - g0)
                w = gn * 128
                dst = (kT if name == "k" else vTx)[
                    0:DK, g0 * 128:g0 * 128 + w]
                transpose_project(
                    name, staged[name][g0:g0 + gn], dst, w, pump, copy_eng)

            # ---- interleaved front half: q/k projections feed scores+exp
            # batches as soon as their operands land; v at the end ----
            pts = [[None] * nbk for _ in range(na)]

            def scores_batch(a, g0):
                for j in range(g0, min(g0 + CG, nbk)):
                    st = ps_st.tile([128, 512], f32, tag="st")
                    nc.tensor.matmul(
                        st[:],
                        kT[:, j * 128:(j + 1) * 128],
                        qT[:, a * 512:(a + 1) * 512],
                    )
                    pt = ptp.tile([128, 512], bf16, tag="pt")
                    nc.scalar.activation(
                        pt[:], st[:],
                        mybir.ActivationFunctionType.Exp,
                        bias=maskb[:, j:j + 1], scale=0.125,
                    )
                    pts[a][j] = pt

            def project_q(a, pump=None, copy_eng=None):
                rows = []
                for ss in range(4):
                    r0 = a * 512 + ss * 128
                    x_sb = kvp.tile([128, DIN], bf16, tag="xqload")
                    nc.sync.dma_start(x_sb[:], xq_d.ap()[r0:r0 + 128, :])
                    rows.append(x_sb)
                transpose_project(
                    "q", rows, qT[:, a * 512:(a + 1) * 512], 512, pump,
                    copy_eng)

            # Round-robin: one projection piece, then up to two ready
            # scores+exp batches, so the st ring / exp pace never stalls
            # the in-order PE queue. V groups interleave early so vaug
            # (and with it the PV phase) starts as soon as v data lands.
            kgr = list(range(0, nbk, CG))
            pieces = []
            for gi, g0 in enumerate(kgr):
                pieces.append(("k", g0))
                if gi < na:
                    pieces.append(("q", gi))
            pieces.extend(("q", a) for a in range(len(kgr), na))
            vpieces = [("v", g0) for g0 in kgr]
            # splice v groups between the later pieces, v-leaning so vaug
            # (gating the PV phase) lands as early as the gathers allow
            merged = pieces[:4]
            rest = pieces[4:]
            if rest and vpieces:
                merged.append(rest.pop(0))
            while vpieces or rest:
                if vpieces:
                    merged.append(vpieces.pop(0))
                    if vpieces and len(vpieces) >= len(rest):
                        merged.append(vpieces.pop(0))
                if rest:
                    merged.append(rest.pop(0))
            pieces = merged

            q_done, k_done = set(), set()
            emitted, backlog = set(), []

            def pump():
                if backlog:
                    scores_batch(*backlog.pop(0))

            for pi, (kind, arg) in enumerate(pieces):
                # late pieces keep their PSUM->SBUF copies off the scalar
                # queue so the exp chain is never delayed behind them
                ce = nc.vector if pi >= 3 else None
                if kind == "q":
                    project_q(arg, pump, ce)
                    q_done.add(arg)
                else:
                    project_kv(kind, arg, pump, ce)
                    if kind == "k":
                        k_done.add(arg)
                for a in sorted(q_done):
                    for gj in sorted(k_done):
                        if (a, gj) not in emitted:
                            emitted.add((a, gj))
                            backlog.append((a, gj))
                pump()
            for a, gj in backlog:
                scores_batch(a, gj)
            nc.sync.dma_start(vaug[:], vTx[:], transpose=True)

            # ---- back half: PV accumulation + output per tile ----
            for a in range(na):
                ot = ps_ot.tile([DK + 2, 512], f32, tag="ot")
                for j in range(nbk):
                    nc.tensor.matmul(
                        ot[:], vaug[:, j, 0:DK + 2], pts[a][j][:],
                        start=(j == 0), stop=(j == nbk - 1),
                    )
                ot_sb = osp.tile([DK + 2, 512], f32r, tag="ot_sb")
                nc.vector.tensor_copy(ot_sb[:], ot[:])
                otp = ps_tpr.tile([128, 4, 128], f32r, tag="tpr")
                o_sb = osp.tile([128, 4, DK], f32, tag="o_sb")
                for ss in range(4):
                    nc.tensor.transpose(
                        otp[:, ss, 0:DK + 2],
                        ot_sb[:, ss * 128:(ss + 1) * 128],
                        identR[:DK + 2, :DK + 2],
                    )
                rcp4 = osp.tile([128, 4], f32, tag="rcp")
                nc.vector.reciprocal(rcp4[:], otp[:, :, DK:DK + 1])
                for ss in range(4):
                    nc.vector.tensor_scalar_mul(
                        o_sb[:, ss, :], otp[:, ss, 0:DK], rcp4[:, ss:ss + 1]
                    )
                r0 = a * 512
                nc.sync.dma_start(
                    out_d.ap()[r0:r0 + 512, :].rearrange(
                        "(c p) e -> p c e", p=128),
                    o_sb[:],
                )

    nc.compile()
    return nc


def _get_nc(s=S, cap=CAP):
    key = (s, cap)
    if key not in _CACHE:
        _CACHE[key] = build_nc(s, cap)
    return _CACHE[key]


def _bf16(a):
    import ml_dtypes
    return np.ascontiguousarray(np.asarray(a).astype(ml_dtypes.bfloat16))


def make_in_maps(query, key, value, mask, W_q, b_q, W_k, b_k, W_v, b_v,
                 cap=CAP):
    """Per-core input dicts. Host work is O(S) metadata plus the bf16
    downcast of the activations/weights (layout prep, like the valid-first
    key permutation derived from the [S] bool mask)."""
    query, key, value = np.asarray(query), np.asarray(key), np.asarray(value)
    mask = np.asarray(mask)
    B = query.shape[0]
    nbk = cap // 128
    wq16, wk16, wv16 = _bf16(W_q), _bf16(W_k), _bf16(W_v)
    in_maps = []
    for b in range(B):
        mrow = mask[b].reshape(-1).astype(bool)
        nvalid = int((~mrow).sum())
        assert nvalid <= cap, f"valid keys {nvalid} exceed CAP={cap}"
        order = np.argsort(mrow, kind="stable")  # valid (False) first
        sel = order[:cap].astype(np.int32)
        kvidx = np.ascontiguousarray(sel.reshape(nbk, 128).T)
        mb = np.where(np.arange(cap) < nvalid, 0.0, MASK_NEG).astype(np.float32)
        maskb = np.ascontiguousarray(mb.reshape(nbk, 128).T)
        in_maps.append({
            "xq": _bf16(query[b]),
            "xk": _bf16(key[b]),
            "xv": _bf16(value[b]),
            "kvidx": kvidx,
            "maskb": maskb,
            "wq": wq16,
            "wk": wk16,
            "wv": wv16,
            "bq": np.ascontiguousarray(np.asarray(b_q).reshape(1, -1)),
            "bk": np.ascontiguousarray(np.asarray(b_k).reshape(1, -1)),
            "bv": np.ascontiguousarray(np.asarray(b_v).reshape(1, -1)),
        })
    return in_maps


def kernel(query, key, value, mask, W_q, b_q, W_k, b_k, W_v, b_v):
    from concourse.bass_utils import run_bass_kernel_spmd

    B = np.asarray(query).shape[0]
    mask_np = np.asarray(mask)
    cap = CAP
    max_valid = max(
        int((~mask_np[b].reshape(-1).astype(bool)).sum()) for b in range(B)
    )
    if max_valid > cap:  # safety net for unexpected masks
        cap = min(S, -(-max_valid // 128) * 128)
    nc = _get_nc(cap=cap)
    in_maps = make_in_maps(query, key, value, mask,
                           W_q, b_q, W_k, b_k, W_v, b_v, cap=cap)
    res = run_bass_kernel_spmd(nc, in_maps, core_ids=list(range(B)))
    out = np.stack([res.results[b]["out"] for b in range(B)], axis=0)
    return out.astype(np.float32)
